# revision 13
# baseline (speedup 1.0000x reference)
"""Committee-of-linear-classifiers vote histogram on 8 Trainium2 cores.

Computation (per sample b):
    logits[m, c] = x[b] . W[m, :, c] + b[m, c]      (16 models, 10 classes)
    vote[m] = argmax_c logits[m, c]
    hist[b, c] = #{m : vote[m] == c}

Strategy (v3):
  - Data-parallel: shard x along batch across the 8 cores (8192 samples each),
    replicate W/b. No cross-device communication.
  - Precision: x and W in SINGLE fp16 (one matmul pass, fp32 PSUM accum).
    Host-measured rel_err of the vote histogram vs the fp32 reference is
    0.0137 (< 2e-2 tolerance). The compare (max + is_ge) runs on the fp32
    logits: an fp16 compare would double-count fp16-grid ties (host-measured
    0.0174 - too close to the limit).
  - Layout: W columns are ordered (class, model) so the one-hot tensor and
    the model-axis histogram sum are fully contiguous on DVE (the fp16 sum
    gets the 2-byte 2x DVE mode; output is fp16, exact for counts <= 16,
    upcast to fp32 on the host).
  - Per 128-sample tile: PE does bias (K=2 ones matmul) + 4 K-chunk fp16
    matmuls -> PSUM [128, 160] fp32; ACT copies PSUM -> SBUF fp32. DVE ops
    are batched per supertile (4 tiles per instruction) to amortize
    per-instruction overhead: reduce_max over classes (strided fp32),
    is_ge vs broadcast max -> fp16 one-hot (contiguous), reduce_sum over
    models (contiguous fp16, 2x) -> [128, 4, 10] fp16.
"""

import os
import sys

import numpy as np

if "/opt/trn_rl_repo" not in sys.path:
    sys.path.insert(0, "/opt/trn_rl_repo")

NCORES = 8
B, D, M, C = 65536, 512, 16, 10
MC = M * C  # 160
BL = B // NCORES  # 8192 samples per core

_NC_CACHE = {}
LAST_RESULT = None  # BassKernelResults of the most recent run (for test harness)


def build_nc(bl=BL, st=512):
    """Build (and compile) the per-core Bass program.

    bl: samples per core, st: samples per supertile (DMA granularity).
    """
    key = (bl, st)
    if key in _NC_CACHE:
        return _NC_CACHE[key]

    from contextlib import ExitStack

    import concourse.bacc as bacc
    import concourse.tile as tile
    from concourse import mybir

    assert bl % st == 0 and st % 128 == 0
    fp16 = mybir.dt.float16
    fp32 = mybir.dt.float32

    nc = bacc.Bacc("TRN2", target_bir_lowering=False, debug=False,
                   enable_asserts=False)
    xh = nc.dram_tensor("xh", [D, bl], fp16, kind="ExternalInput").ap()
    wh = nc.dram_tensor("wh", [D, MC], fp16, kind="ExternalInput").ap()
    bhl = nc.dram_tensor("bhl", [2, MC], fp16, kind="ExternalInput").ap()
    out = nc.dram_tensor("out", [bl, C], fp32, kind="ExternalOutput").ap()

    KCH = D // 128  # 4 contraction chunks

    with tile.TileContext(nc) as tc, ExitStack() as ctx:
        wpool = ctx.enter_context(tc.tile_pool(name="wpool", bufs=1))
        xpool = ctx.enter_context(tc.tile_pool(name="xpool", bufs=4))
        ppool = ctx.enter_context(tc.tile_pool(name="ppool", bufs=8, space="PSUM"))
        tpool = ctx.enter_context(tc.tile_pool(name="tpool", bufs=4))
        gpool = ctx.enter_context(tc.tile_pool(name="gpool", bufs=4))
        mpool = ctx.enter_context(tc.tile_pool(name="mpool", bufs=4))
        opool = ctx.enter_context(tc.tile_pool(name="opool", bufs=3))

        xh_r = xh.rearrange("(k p) b -> p k b", p=128)
        wh_r = wh.rearrange("(k p) n -> p k n", p=128)

        # First x slice goes out first (it gates the first matmul), W is
        # split per chunk so chunk 0 lands quickly, bias goes in parallel.
        xh_t0 = xpool.tile([128, KCH, st], fp16)
        nc.sync.dma_start(xh_t0[:, :, 0:128], xh_r[:, :, 0:128])
        whs = wpool.tile([128, KCH, MC], fp16)
        for k in range(KCH):
            nc.scalar.dma_start(whs[:, k, :], wh_r[:, k, :])
        bs = wpool.tile([2, MC], fp16)
        nc.scalar.dma_start(bs, bhl)
        ones2 = wpool.tile([2, 128], fp16)
        nc.gpsimd.memset(ones2, 1.0)

        for s in range(bl // st):
            if s == 0:
                xh_t = xh_t0
                nc.sync.dma_start(xh_t[:, :, 128:st], xh_r[:, :, 128:st])
            else:
                xh_t = xpool.tile([128, KCH, st], fp16)
                nc.sync.dma_start(xh_t, xh_r[:, :, s * st:(s + 1) * st])
            nj = st // 128
            outst = opool.tile([128, nj, C], fp32)
            t = tpool.tile([128, nj, MC], fp32)
            for j in range(nj):
                bsl = slice(j * 128, (j + 1) * 128)
                ps = ppool.tile([128, MC], fp32)
                # bias matmul last: the first chunk matmul only needs x+W,
                # so the PE can start before the bias DMA lands
                for k in range(KCH):
                    nc.tensor.matmul(ps, lhsT=xh_t[:, k, bsl], rhs=whs[:, k, :],
                                     start=(k == 0), stop=False)
                nc.tensor.matmul(ps, lhsT=ones2, rhs=bs, start=False, stop=True)
                # logits tile -> SBUF (ACT) at full fp32
                nc.scalar.copy(t[:, j, :], ps)
            # Batched DVE ops over the whole supertile (nj tiles per
            # instruction) to amortize per-instruction overheads.
            # Storage order within a tile is (c, m): index = c*M + m.
            # 3-dim APs where possible (4-dim reduce APs hit a slow path).
            # per-model max over the 10 classes (strided fp32 reads)
            mx = mpool.tile([128, nj, M], fp32)
            nc.vector.reduce_max(mx, t.rearrange("p j (c m) -> p j m c", m=M),
                                 axis=mybir.AxisListType.X)
            # one-hot votes, fully contiguous write [p, j, c, m]. fp32: DVE
            # reads 2-byte data at ~half rate (no 2x mode engages), so fp32
            # end-to-end is faster here.
            ge = gpool.tile([128, nj, C, M], fp32)
            nc.vector.tensor_tensor(
                ge,
                t.rearrange("p j (c m) -> p j c m", m=M),
                mx.unsqueeze(2).broadcast_to((128, nj, C, M)),
                mybir.AluOpType.is_ge)
            # histogram: sum over the (contiguous) model axis -> [128, nj, 10]
            nc.vector.reduce_sum(outst.rearrange("p j c -> p (j c)"),
                                 ge.rearrange("p j c m -> p (j c) m"),
                                 axis=mybir.AxisListType.X)
            # out-DMA triggers go on the otherwise-idle Pool queue: on the
            # Scalar queue they would head-of-line block the next
            # supertile's ACT copies behind the DVE sum.
            orr = out[s * st:(s + 1) * st, :].rearrange("(j p) c -> p j c", p=128)
            if s == bl // st - 1:
                # split the last supertile's output so the final (tail-
                # critical) DMA is small
                half = st // 256
                nc.gpsimd.dma_start(orr[:, :half, :], outst[:, :half, :])
                nc.gpsimd.dma_start(orr[:, half:, :], outst[:, half:, :])
            else:
                nc.gpsimd.dma_start(orr, outst)

    nc.compile()
    _NC_CACHE[key] = nc
    return nc


def make_in_maps(x, W, b, ncores=NCORES):
    """Host-side prep: transpose + fp16 cast + per-core sharding.

    W/bias columns are laid out in (class, model) order: index = c*M + m.
    """
    x = np.asarray(x, dtype=np.float32)
    W = np.asarray(W, dtype=np.float32)
    b = np.asarray(b, dtype=np.float32)

    xT = np.ascontiguousarray(x.T)                      # [D, B]
    xh = xT.astype(np.float16)

    Wt = np.ascontiguousarray(W.transpose(1, 2, 0).reshape(D, MC))  # [D, (c m)]
    wh16 = Wt.astype(np.float16)

    bf = np.ascontiguousarray(b.T.reshape(MC))          # [(c m)]
    bh = bf.astype(np.float16)
    bl16 = (bf - bh.astype(np.float32)).astype(np.float16)
    bhl = np.ascontiguousarray(np.stack([bh, bl16]))    # [2, 160]

    bl_sz = x.shape[0] // ncores
    in_maps = []
    for c in range(ncores):
        sl = slice(c * bl_sz, (c + 1) * bl_sz)
        in_maps.append({
            "xh": np.ascontiguousarray(xh[:, sl]),
            "wh": wh16,
            "bhl": bhl,
        })
    return in_maps


def kernel(x, W, b):
    global LAST_RESULT
    from concourse import bass_utils

    # NTFF tracing under axon needs the antenv.axon_hooks shim; without it
    # run_bass_kernel_spmd(trace=True) raises. Disable tracing defensively
    # when the hook module is absent (BASS_TRACE may be set in the env).
    want_trace = bool(os.environ.get("BASS_TRACE"))
    try:
        from antenv.axon_hooks import get_axon_ntff_profile_hook  # noqa: F401
    except ImportError:
        want_trace = False
        os.environ["BASS_NEVER_TRACE"] = "1"

    in_maps = make_in_maps(x, W, b)
    nc = build_nc(BL, 512)
    res = bass_utils.run_bass_kernel_spmd(
        nc, in_maps, core_ids=list(range(NCORES)),
        trace=want_trace,
    )
    LAST_RESULT = res
    return np.concatenate([r["out"] for r in res.results], axis=0)


# revision 14
# speedup vs baseline: 1.0901x; 1.0901x over previous
"""Committee-of-linear-classifiers vote histogram on 8 Trainium2 cores.

Computation (per sample b):
    logits[m, c] = x[b] . W[m, :, c] + b[m, c]      (16 models, 10 classes)
    vote[m] = argmax_c logits[m, c]
    hist[b, c] = #{m : vote[m] == c}

Strategy (v3):
  - Data-parallel: shard x along batch across the 8 cores (8192 samples each),
    replicate W/b. No cross-device communication.
  - Precision: x and W in SINGLE fp16 (one matmul pass, fp32 PSUM accum).
    Host-measured rel_err of the vote histogram vs the fp32 reference is
    0.0137 (< 2e-2 tolerance). The compare (max + is_ge) runs on the fp32
    logits: an fp16 compare would double-count fp16-grid ties (host-measured
    0.0174 - too close to the limit).
  - Layout: W columns are ordered (class, model) so the one-hot tensor and
    the model-axis histogram sum are fully contiguous on DVE (the fp16 sum
    gets the 2-byte 2x DVE mode; output is fp16, exact for counts <= 16,
    upcast to fp32 on the host).
  - Per 128-sample tile: PE does bias (K=2 ones matmul) + 4 K-chunk fp16
    matmuls -> PSUM [128, 160] fp32; ACT copies PSUM -> SBUF fp32. DVE ops
    are batched per supertile (4 tiles per instruction) to amortize
    per-instruction overhead: reduce_max over classes (strided fp32),
    is_ge vs broadcast max -> fp16 one-hot (contiguous), reduce_sum over
    models (contiguous fp16, 2x) -> [128, 4, 10] fp16.
"""

import os
import sys

import numpy as np

if "/opt/trn_rl_repo" not in sys.path:
    sys.path.insert(0, "/opt/trn_rl_repo")

NCORES = 8
B, D, M, C = 65536, 512, 16, 10
MC = M * C  # 160
BL = B // NCORES  # 8192 samples per core

_NC_CACHE = {}
LAST_RESULT = None  # BassKernelResults of the most recent run (for test harness)


def build_nc(bl=BL, st=512):
    """Build (and compile) the per-core Bass program.

    bl: samples per core, st: samples per supertile (DMA granularity).
    """
    key = (bl, st)
    if key in _NC_CACHE:
        return _NC_CACHE[key]

    from contextlib import ExitStack

    import concourse.bacc as bacc
    import concourse.tile as tile
    from concourse import mybir

    assert bl % st == 0 and st % 128 == 0
    fp16 = mybir.dt.float16
    fp32 = mybir.dt.float32

    nc = bacc.Bacc("TRN2", target_bir_lowering=False, debug=False,
                   enable_asserts=False)
    xh = nc.dram_tensor("xh", [D, bl], fp16, kind="ExternalInput").ap()
    wh = nc.dram_tensor("wh", [D, MC], fp16, kind="ExternalInput").ap()
    bhl = nc.dram_tensor("bhl", [2, MC], fp16, kind="ExternalInput").ap()
    out = nc.dram_tensor("out", [bl, C], fp32, kind="ExternalOutput").ap()

    KCH = D // 128  # 4 contraction chunks

    with tile.TileContext(nc) as tc, ExitStack() as ctx:
        wpool = ctx.enter_context(tc.tile_pool(name="wpool", bufs=1))
        xpool = ctx.enter_context(tc.tile_pool(name="xpool", bufs=4))
        ppool = ctx.enter_context(tc.tile_pool(name="ppool", bufs=8, space="PSUM"))
        tpool = ctx.enter_context(tc.tile_pool(name="tpool", bufs=4))
        gpool = ctx.enter_context(tc.tile_pool(name="gpool", bufs=4))
        mpool = ctx.enter_context(tc.tile_pool(name="mpool", bufs=4))
        opool = ctx.enter_context(tc.tile_pool(name="opool", bufs=3))

        xh_r = xh.rearrange("(k p) b -> p k b", p=128)
        wh_r = wh.rearrange("(k p) n -> p k n", p=128)

        # First x slice goes out first (it gates the first matmul), W is
        # split per chunk so chunk 0 lands quickly, bias goes in parallel.
        xh_t0 = xpool.tile([128, KCH, st], fp16)
        nc.sync.dma_start(xh_t0[:, :, 0:128], xh_r[:, :, 0:128])
        whs = wpool.tile([128, KCH, MC], fp16)
        for k in range(KCH):
            nc.scalar.dma_start(whs[:, k, :], wh_r[:, k, :])
        bs = wpool.tile([2, MC], fp16)
        nc.scalar.dma_start(bs, bhl)
        ones2 = wpool.tile([2, 128], fp16)
        nc.gpsimd.memset(ones2, 1.0)

        for s in range(bl // st):
            if s == 0:
                xh_t = xh_t0
                nc.sync.dma_start(xh_t[:, :, 128:st], xh_r[:, :, 128:st])
            else:
                xh_t = xpool.tile([128, KCH, st], fp16)
                nc.sync.dma_start(xh_t, xh_r[:, :, s * st:(s + 1) * st])
            nj = st // 128
            outst = opool.tile([128, nj, C], fp32)
            t = tpool.tile([128, nj, MC], fp32)
            for j in range(nj):
                bsl = slice(j * 128, (j + 1) * 128)
                ps = ppool.tile([128, MC], fp32)
                nc.tensor.matmul(ps, lhsT=ones2, rhs=bs, start=True, stop=False)
                for k in range(KCH):
                    nc.tensor.matmul(ps, lhsT=xh_t[:, k, bsl], rhs=whs[:, k, :],
                                     start=False, stop=(k == KCH - 1))
                # logits tile -> SBUF (ACT) at full fp32
                nc.scalar.copy(t[:, j, :], ps)
            # Batched DVE ops over the whole supertile (nj tiles per
            # instruction) to amortize per-instruction overheads.
            # Storage order within a tile is (c, m): index = c*M + m.
            # 3-dim APs where possible (4-dim reduce APs hit a slow path).
            # per-model max over the 10 classes (strided fp32 reads)
            mx = mpool.tile([128, nj, M], fp32)
            nc.vector.reduce_max(mx, t.rearrange("p j (c m) -> p j m c", m=M),
                                 axis=mybir.AxisListType.X)
            # one-hot votes, fully contiguous write [p, j, c, m]. fp32: DVE
            # reads 2-byte data at ~half rate (no 2x mode engages), so fp32
            # end-to-end is faster here.
            ge = gpool.tile([128, nj, C, M], fp32)
            nc.vector.tensor_tensor(
                ge,
                t.rearrange("p j (c m) -> p j c m", m=M),
                mx.unsqueeze(2).broadcast_to((128, nj, C, M)),
                mybir.AluOpType.is_ge)
            # histogram: sum over the (contiguous) model axis -> [128, nj, 10]
            nc.vector.reduce_sum(outst.rearrange("p j c -> p (j c)"),
                                 ge.rearrange("p j c m -> p (j c) m"),
                                 axis=mybir.AxisListType.X)
            # out-DMA triggers go on the otherwise-idle Pool queue: on the
            # Scalar queue they would head-of-line block the next
            # supertile's ACT copies behind the DVE sum.
            orr = out[s * st:(s + 1) * st, :].rearrange("(j p) c -> p j c", p=128)
            if s == bl // st - 1:
                # split the last supertile's output so the final (tail-
                # critical) DMA is small
                half = st // 256
                nc.gpsimd.dma_start(orr[:, :half, :], outst[:, :half, :])
                nc.gpsimd.dma_start(orr[:, half:, :], outst[:, half:, :])
            else:
                nc.gpsimd.dma_start(orr, outst)

    nc.compile()
    _NC_CACHE[key] = nc
    return nc


def make_in_maps(x, W, b, ncores=NCORES):
    """Host-side prep: transpose + fp16 cast + per-core sharding.

    W/bias columns are laid out in (class, model) order: index = c*M + m.
    """
    x = np.asarray(x, dtype=np.float32)
    W = np.asarray(W, dtype=np.float32)
    b = np.asarray(b, dtype=np.float32)

    xT = np.ascontiguousarray(x.T)                      # [D, B]
    xh = xT.astype(np.float16)

    Wt = np.ascontiguousarray(W.transpose(1, 2, 0).reshape(D, MC))  # [D, (c m)]
    wh16 = Wt.astype(np.float16)

    bf = np.ascontiguousarray(b.T.reshape(MC))          # [(c m)]
    bh = bf.astype(np.float16)
    bl16 = (bf - bh.astype(np.float32)).astype(np.float16)
    bhl = np.ascontiguousarray(np.stack([bh, bl16]))    # [2, 160]

    bl_sz = x.shape[0] // ncores
    in_maps = []
    for c in range(ncores):
        sl = slice(c * bl_sz, (c + 1) * bl_sz)
        in_maps.append({
            "xh": np.ascontiguousarray(xh[:, sl]),
            "wh": wh16,
            "bhl": bhl,
        })
    return in_maps


def kernel(x, W, b):
    global LAST_RESULT
    from concourse import bass_utils

    # NTFF tracing under axon needs the antenv.axon_hooks shim; without it
    # run_bass_kernel_spmd(trace=True) raises. Disable tracing defensively
    # when the hook module is absent (BASS_TRACE may be set in the env).
    want_trace = bool(os.environ.get("BASS_TRACE"))
    try:
        from antenv.axon_hooks import get_axon_ntff_profile_hook  # noqa: F401
    except ImportError:
        want_trace = False
        os.environ["BASS_NEVER_TRACE"] = "1"

    in_maps = make_in_maps(x, W, b)
    nc = build_nc(BL, 512)
    res = bass_utils.run_bass_kernel_spmd(
        nc, in_maps, core_ids=list(range(NCORES)),
        trace=want_trace,
    )
    LAST_RESULT = res
    return np.concatenate([r["out"] for r in res.results], axis=0)


# revision 15
# speedup vs baseline: 1.1201x; 1.0275x over previous
"""Committee-of-linear-classifiers vote histogram on 8 Trainium2 cores.

Computation (per sample b):
    logits[m, c] = x[b] . W[m, :, c] + b[m, c]      (16 models, 10 classes)
    vote[m] = argmax_c logits[m, c]
    hist[b, c] = #{m : vote[m] == c}

Strategy (v3):
  - Data-parallel: shard x along batch across the 8 cores (8192 samples each),
    replicate W/b. No cross-device communication.
  - Precision: x and W in SINGLE fp16 (one matmul pass, fp32 PSUM accum).
    Host-measured rel_err of the vote histogram vs the fp32 reference is
    0.0137 (< 2e-2 tolerance). The compare (max + is_ge) runs on the fp32
    logits: an fp16 compare would double-count fp16-grid ties (host-measured
    0.0174 - too close to the limit).
  - Layout: W columns are ordered (class, model) so the one-hot tensor and
    the model-axis histogram sum are fully contiguous on DVE (the fp16 sum
    gets the 2-byte 2x DVE mode; output is fp16, exact for counts <= 16,
    upcast to fp32 on the host).
  - Per 128-sample tile: PE does bias (K=2 ones matmul) + 4 K-chunk fp16
    matmuls -> PSUM [128, 160] fp32; ACT copies PSUM -> SBUF fp32. DVE ops
    are batched per supertile (4 tiles per instruction) to amortize
    per-instruction overhead: reduce_max over classes (strided fp32),
    is_ge vs broadcast max -> fp16 one-hot (contiguous), reduce_sum over
    models (contiguous fp16, 2x) -> [128, 4, 10] fp16.
"""

import os
import sys

import numpy as np

if "/opt/trn_rl_repo" not in sys.path:
    sys.path.insert(0, "/opt/trn_rl_repo")

NCORES = 8
B, D, M, C = 65536, 512, 16, 10
MC = M * C  # 160
BL = B // NCORES  # 8192 samples per core

_NC_CACHE = {}
LAST_RESULT = None  # BassKernelResults of the most recent run (for test harness)


def build_nc(bl=BL, st=512):
    """Build (and compile) the per-core Bass program.

    bl: samples per core, st: samples per supertile (DMA granularity).
    """
    key = (bl, st)
    if key in _NC_CACHE:
        return _NC_CACHE[key]

    from contextlib import ExitStack

    import concourse.bacc as bacc
    import concourse.tile as tile
    from concourse import mybir

    assert bl % st == 0 and st % 128 == 0
    fp16 = mybir.dt.float16
    fp32 = mybir.dt.float32

    nc = bacc.Bacc("TRN2", target_bir_lowering=False, debug=False,
                   enable_asserts=False)
    xh = nc.dram_tensor("xh", [D, bl], fp16, kind="ExternalInput").ap()
    wh = nc.dram_tensor("wh", [D, MC], fp16, kind="ExternalInput").ap()
    bhl = nc.dram_tensor("bhl", [2, MC], fp16, kind="ExternalInput").ap()
    out = nc.dram_tensor("out", [bl, C], fp32, kind="ExternalOutput").ap()

    KCH = D // 128  # 4 contraction chunks

    with tile.TileContext(nc) as tc, ExitStack() as ctx:
        wpool = ctx.enter_context(tc.tile_pool(name="wpool", bufs=1))
        xpool = ctx.enter_context(tc.tile_pool(name="xpool", bufs=4))
        ppool = ctx.enter_context(tc.tile_pool(name="ppool", bufs=8, space="PSUM"))
        tpool = ctx.enter_context(tc.tile_pool(name="tpool", bufs=4))
        gpool = ctx.enter_context(tc.tile_pool(name="gpool", bufs=4))
        mpool = ctx.enter_context(tc.tile_pool(name="mpool", bufs=4))
        opool = ctx.enter_context(tc.tile_pool(name="opool", bufs=3))

        xh_r = xh.rearrange("(k p) b -> p k b", p=128)
        wh_r = wh.rearrange("(k p) n -> p k n", p=128)

        # Issue order matters: the first matmul is the bias matmul, so the
        # (tiny) bias DMA and ones memset go absolutely first, then the
        # first x slice, then W split per chunk so chunk 0 lands quickly.
        ones2 = wpool.tile([2, 128], fp16)
        nc.gpsimd.memset(ones2, 1.0)
        bs = wpool.tile([2, MC], fp16)
        nc.scalar.dma_start(bs, bhl)
        xh_t0 = xpool.tile([128, KCH, st], fp16)
        nc.sync.dma_start(xh_t0[:, :, 0:128], xh_r[:, :, 0:128])
        whs = wpool.tile([128, KCH, MC], fp16)
        for k in range(KCH):
            nc.scalar.dma_start(whs[:, k, :], wh_r[:, k, :])

        for s in range(bl // st):
            if s == 0:
                xh_t = xh_t0
                nc.sync.dma_start(xh_t[:, :, 128:st], xh_r[:, :, 128:st])
            else:
                xh_t = xpool.tile([128, KCH, st], fp16)
                nc.sync.dma_start(xh_t, xh_r[:, :, s * st:(s + 1) * st])
            nj = st // 128
            outst = opool.tile([128, nj, C], fp32)
            t = tpool.tile([128, nj, MC], fp32)
            for j in range(nj):
                bsl = slice(j * 128, (j + 1) * 128)
                ps = ppool.tile([128, MC], fp32)
                nc.tensor.matmul(ps, lhsT=ones2, rhs=bs, start=True, stop=False)
                for k in range(KCH):
                    nc.tensor.matmul(ps, lhsT=xh_t[:, k, bsl], rhs=whs[:, k, :],
                                     start=False, stop=(k == KCH - 1))
                # logits tile -> SBUF (ACT) at full fp32
                nc.scalar.copy(t[:, j, :], ps)
            # Batched DVE ops over the whole supertile (nj tiles per
            # instruction) to amortize per-instruction overheads.
            # Storage order within a tile is (c, m): index = c*M + m.
            # 3-dim APs where possible (4-dim reduce APs hit a slow path).
            # per-model max over the 10 classes (strided fp32 reads)
            mx = mpool.tile([128, nj, M], fp32)
            nc.vector.reduce_max(mx, t.rearrange("p j (c m) -> p j m c", m=M),
                                 axis=mybir.AxisListType.X)
            # one-hot votes, fully contiguous write [p, j, c, m]. fp32: DVE
            # reads 2-byte data at ~half rate (no 2x mode engages), so fp32
            # end-to-end is faster here.
            ge = gpool.tile([128, nj, C, M], fp32)
            nc.vector.tensor_tensor(
                ge,
                t.rearrange("p j (c m) -> p j c m", m=M),
                mx.unsqueeze(2).broadcast_to((128, nj, C, M)),
                mybir.AluOpType.is_ge)
            # histogram: sum over the (contiguous) model axis -> [128, nj, 10]
            nc.vector.reduce_sum(outst.rearrange("p j c -> p (j c)"),
                                 ge.rearrange("p j c m -> p (j c) m"),
                                 axis=mybir.AxisListType.X)
            # out-DMA triggers go on the otherwise-idle Pool queue: on the
            # Scalar queue they would head-of-line block the next
            # supertile's ACT copies behind the DVE sum.
            orr = out[s * st:(s + 1) * st, :].rearrange("(j p) c -> p j c", p=128)
            if s == bl // st - 1:
                # split the last supertile's output so the final (tail-
                # critical) DMA is small
                half = st // 256
                nc.gpsimd.dma_start(orr[:, :half, :], outst[:, :half, :])
                nc.gpsimd.dma_start(orr[:, half:, :], outst[:, half:, :])
            else:
                nc.gpsimd.dma_start(orr, outst)

    nc.compile()
    _NC_CACHE[key] = nc
    return nc


def make_in_maps(x, W, b, ncores=NCORES):
    """Host-side prep: transpose + fp16 cast + per-core sharding.

    W/bias columns are laid out in (class, model) order: index = c*M + m.
    """
    x = np.asarray(x, dtype=np.float32)
    W = np.asarray(W, dtype=np.float32)
    b = np.asarray(b, dtype=np.float32)

    xT = np.ascontiguousarray(x.T)                      # [D, B]
    xh = xT.astype(np.float16)

    Wt = np.ascontiguousarray(W.transpose(1, 2, 0).reshape(D, MC))  # [D, (c m)]
    wh16 = Wt.astype(np.float16)

    bf = np.ascontiguousarray(b.T.reshape(MC))          # [(c m)]
    bh = bf.astype(np.float16)
    bl16 = (bf - bh.astype(np.float32)).astype(np.float16)
    bhl = np.ascontiguousarray(np.stack([bh, bl16]))    # [2, 160]

    bl_sz = x.shape[0] // ncores
    in_maps = []
    for c in range(ncores):
        sl = slice(c * bl_sz, (c + 1) * bl_sz)
        in_maps.append({
            "xh": np.ascontiguousarray(xh[:, sl]),
            "wh": wh16,
            "bhl": bhl,
        })
    return in_maps


def kernel(x, W, b):
    global LAST_RESULT
    from concourse import bass_utils

    # NTFF tracing under axon needs the antenv.axon_hooks shim; without it
    # run_bass_kernel_spmd(trace=True) raises. Disable tracing defensively
    # when the hook module is absent (BASS_TRACE may be set in the env).
    want_trace = bool(os.environ.get("BASS_TRACE"))
    try:
        from antenv.axon_hooks import get_axon_ntff_profile_hook  # noqa: F401
    except ImportError:
        want_trace = False
        os.environ["BASS_NEVER_TRACE"] = "1"

    in_maps = make_in_maps(x, W, b)
    nc = build_nc(BL, 512)
    res = bass_utils.run_bass_kernel_spmd(
        nc, in_maps, core_ids=list(range(NCORES)),
        trace=want_trace,
    )
    LAST_RESULT = res
    return np.concatenate([r["out"] for r in res.results], axis=0)


# revision 16
# speedup vs baseline: 1.1858x; 1.0587x over previous
"""Committee-of-linear-classifiers vote histogram on 8 Trainium2 cores.

Computation (per sample b):
    logits[m, c] = x[b] . W[m, :, c] + b[m, c]      (16 models, 10 classes)
    vote[m] = argmax_c logits[m, c]
    hist[b, c] = #{m : vote[m] == c}

Strategy (v3):
  - Data-parallel: shard x along batch across the 8 cores (8192 samples each),
    replicate W/b. No cross-device communication.
  - Precision: x and W in SINGLE fp16 (one matmul pass, fp32 PSUM accum).
    Host-measured rel_err of the vote histogram vs the fp32 reference is
    0.0137 (< 2e-2 tolerance). The compare (max + is_ge) runs on the fp32
    logits: an fp16 compare would double-count fp16-grid ties (host-measured
    0.0174 - too close to the limit).
  - Layout: W columns are ordered (class, model) so the one-hot tensor and
    the model-axis histogram sum are fully contiguous on DVE (the fp16 sum
    gets the 2-byte 2x DVE mode; output is fp16, exact for counts <= 16,
    upcast to fp32 on the host).
  - Per 128-sample tile: PE does bias (K=2 ones matmul) + 4 K-chunk fp16
    matmuls -> PSUM [128, 160] fp32; ACT copies PSUM -> SBUF fp32. DVE ops
    are batched per supertile (4 tiles per instruction) to amortize
    per-instruction overhead: reduce_max over classes (strided fp32),
    is_ge vs broadcast max -> fp16 one-hot (contiguous), reduce_sum over
    models (contiguous fp16, 2x) -> [128, 4, 10] fp16.
"""

import os
import sys

import numpy as np

if "/opt/trn_rl_repo" not in sys.path:
    sys.path.insert(0, "/opt/trn_rl_repo")

NCORES = 8
B, D, M, C = 65536, 512, 16, 10
MC = M * C  # 160
BL = B // NCORES  # 8192 samples per core

_NC_CACHE = {}
LAST_RESULT = None  # BassKernelResults of the most recent run (for test harness)


def build_nc(bl=BL, st=512):
    """Build (and compile) the per-core Bass program.

    bl: samples per core, st: samples per supertile (DMA granularity).
    """
    key = (bl, st)
    if key in _NC_CACHE:
        return _NC_CACHE[key]

    from contextlib import ExitStack

    import concourse.bacc as bacc
    import concourse.tile as tile
    from concourse import mybir

    assert bl % st == 0 and st % 128 == 0
    fp16 = mybir.dt.float16
    fp32 = mybir.dt.float32

    nc = bacc.Bacc("TRN2", target_bir_lowering=False, debug=False,
                   enable_asserts=False)
    xh = nc.dram_tensor("xh", [D, bl], fp16, kind="ExternalInput").ap()
    wh = nc.dram_tensor("wh", [D, MC], fp16, kind="ExternalInput").ap()
    bhl = nc.dram_tensor("bhl", [2, MC], fp16, kind="ExternalInput").ap()
    out = nc.dram_tensor("out", [bl, C], fp32, kind="ExternalOutput").ap()

    KCH = D // 128  # 4 contraction chunks

    with tile.TileContext(nc) as tc, ExitStack() as ctx:
        wpool = ctx.enter_context(tc.tile_pool(name="wpool", bufs=1))
        xpool = ctx.enter_context(tc.tile_pool(name="xpool", bufs=6))
        ppool = ctx.enter_context(tc.tile_pool(name="ppool", bufs=8, space="PSUM"))
        tpool = ctx.enter_context(tc.tile_pool(name="tpool", bufs=4))
        gpool = ctx.enter_context(tc.tile_pool(name="gpool", bufs=4))
        mpool = ctx.enter_context(tc.tile_pool(name="mpool", bufs=4))
        opool = ctx.enter_context(tc.tile_pool(name="opool", bufs=3))

        xh_r = xh.rearrange("(k p) b -> p k b", p=128)
        wh_r = wh.rearrange("(k p) n -> p k n", p=128)

        # Issue order matters: the first matmul is the bias matmul, so the
        # (tiny) bias DMA and ones memset go absolutely first, then the
        # first x slice, then W split per chunk so chunk 0 lands quickly.
        ones2 = wpool.tile([2, 128], fp16)
        nc.gpsimd.memset(ones2, 1.0)
        bs = wpool.tile([2, MC], fp16)
        nc.scalar.dma_start(bs, bhl)
        xh_t0 = xpool.tile([128, KCH, st], fp16)
        nc.sync.dma_start(xh_t0[:, :, 0:128], xh_r[:, :, 0:128])
        whs = wpool.tile([128, KCH, MC], fp16)
        for k in range(KCH):
            nc.scalar.dma_start(whs[:, k, :], wh_r[:, k, :])

        for s in range(bl // st):
            if s == 0:
                xh_t = xh_t0
                nc.sync.dma_start(xh_t[:, :, 128:st], xh_r[:, :, 128:st])
            else:
                xh_t = xpool.tile([128, KCH, st], fp16)
                nc.sync.dma_start(xh_t, xh_r[:, :, s * st:(s + 1) * st])
            nj = st // 128
            outst = opool.tile([128, nj, C], fp32)
            t = tpool.tile([128, nj, MC], fp32)
            for j in range(nj):
                bsl = slice(j * 128, (j + 1) * 128)
                ps = ppool.tile([128, MC], fp32)
                nc.tensor.matmul(ps, lhsT=ones2, rhs=bs, start=True, stop=False)
                for k in range(KCH):
                    nc.tensor.matmul(ps, lhsT=xh_t[:, k, bsl], rhs=whs[:, k, :],
                                     start=False, stop=(k == KCH - 1))
                # logits tile -> SBUF (ACT) at full fp32
                nc.scalar.copy(t[:, j, :], ps)
            # Batched DVE ops over the whole supertile (nj tiles per
            # instruction) to amortize per-instruction overheads.
            # Storage order within a tile is (c, m): index = c*M + m.
            # 3-dim APs where possible (4-dim reduce APs hit a slow path).
            # per-model max over the 10 classes (strided fp32 reads)
            mx = mpool.tile([128, nj, M], fp32)
            nc.vector.reduce_max(mx, t.rearrange("p j (c m) -> p j m c", m=M),
                                 axis=mybir.AxisListType.X)
            # one-hot votes, fully contiguous write [p, j, c, m]. fp32: DVE
            # reads 2-byte data at ~half rate (no 2x mode engages), so fp32
            # end-to-end is faster here.
            ge = gpool.tile([128, nj, C, M], fp32)
            nc.vector.tensor_tensor(
                ge,
                t.rearrange("p j (c m) -> p j c m", m=M),
                mx.unsqueeze(2).broadcast_to((128, nj, C, M)),
                mybir.AluOpType.is_ge)
            # histogram: sum over the (contiguous) model axis -> [128, nj, 10]
            nc.vector.reduce_sum(outst.rearrange("p j c -> p (j c)"),
                                 ge.rearrange("p j c m -> p (j c) m"),
                                 axis=mybir.AxisListType.X)
            # out-DMA triggers go on the otherwise-idle Pool queue: on the
            # Scalar queue they would head-of-line block the next
            # supertile's ACT copies behind the DVE sum.
            orr = out[s * st:(s + 1) * st, :].rearrange("(j p) c -> p j c", p=128)
            if s == bl // st - 1:
                # split the last supertile's output so the final (tail-
                # critical) DMA is small
                half = st // 256
                nc.gpsimd.dma_start(orr[:, :half, :], outst[:, :half, :])
                nc.gpsimd.dma_start(orr[:, half:, :], outst[:, half:, :])
            else:
                nc.gpsimd.dma_start(orr, outst)

    nc.compile()
    _NC_CACHE[key] = nc
    return nc


def make_in_maps(x, W, b, ncores=NCORES):
    """Host-side prep: transpose + fp16 cast + per-core sharding.

    W/bias columns are laid out in (class, model) order: index = c*M + m.
    """
    x = np.asarray(x, dtype=np.float32)
    W = np.asarray(W, dtype=np.float32)
    b = np.asarray(b, dtype=np.float32)

    xT = np.ascontiguousarray(x.T)                      # [D, B]
    xh = xT.astype(np.float16)

    Wt = np.ascontiguousarray(W.transpose(1, 2, 0).reshape(D, MC))  # [D, (c m)]
    wh16 = Wt.astype(np.float16)

    bf = np.ascontiguousarray(b.T.reshape(MC))          # [(c m)]
    bh = bf.astype(np.float16)
    bl16 = (bf - bh.astype(np.float32)).astype(np.float16)
    bhl = np.ascontiguousarray(np.stack([bh, bl16]))    # [2, 160]

    bl_sz = x.shape[0] // ncores
    in_maps = []
    for c in range(ncores):
        sl = slice(c * bl_sz, (c + 1) * bl_sz)
        in_maps.append({
            "xh": np.ascontiguousarray(xh[:, sl]),
            "wh": wh16,
            "bhl": bhl,
        })
    return in_maps


def kernel(x, W, b):
    global LAST_RESULT
    from concourse import bass_utils

    # NTFF tracing under axon needs the antenv.axon_hooks shim; without it
    # run_bass_kernel_spmd(trace=True) raises. Disable tracing defensively
    # when the hook module is absent (BASS_TRACE may be set in the env).
    want_trace = bool(os.environ.get("BASS_TRACE"))
    try:
        from antenv.axon_hooks import get_axon_ntff_profile_hook  # noqa: F401
    except ImportError:
        want_trace = False
        os.environ["BASS_NEVER_TRACE"] = "1"

    in_maps = make_in_maps(x, W, b)
    nc = build_nc(BL, 512)
    res = bass_utils.run_bass_kernel_spmd(
        nc, in_maps, core_ids=list(range(NCORES)),
        trace=want_trace,
    )
    LAST_RESULT = res
    return np.concatenate([r["out"] for r in res.results], axis=0)
